# revision 28
# baseline (speedup 1.0000x reference)
"""Biaffine edge attention on 8 Trainium2 NeuronCores.

Math (per batch b):
    out[i,o] = head[i,:] @ U @ dep[o,:] + head[i,:]@wh + dep[o,:]@wd + b
with head/dep [S=2048, D=256], U [D,D], edge_W = [wh | wd] (each [D]).

Sharding: pure data-parallel over batch B=8 -> one batch per core,
U / edge_W / edge_b replicated. No collectives.

Per-core kernel:
    ATf[e,i] = sum_d U[d,e] * headT[d,i] + wd[e]      (dep-side rank-1 term
               rides the e-contraction for free)
    hs[i]    = sum_d head[i,d] * wh[d]  + b           (DVE mul+reduce)
    out[i,o] = sum_e ATf[e,i] * depT[e,o]  + hs[i]

All 8 cores run in phase, so each NC sees ~358 GB/s (HBM stack shared
with its neighbor) and exec_time ~= first_store_time + 16.8MB/358GB/s.
The schedule therefore minimizes time-to-first-store and keeps the
store ring saturated:
  * Load order (all triggers upfront on the scalar/ACT HWDGE ring):
    consts, head g0, dep g0..g3, head g1..g3.  With head g0 first, the
    ATf/hs chain for out rows 0..511 completes while dep g0 is still
    loading, and the only work READY in the early window is the
    critical chain -- load arrival itself gates all distractor work,
    so the greedy Tile scheduler cannot jam the chain.
  * Out rows 0..511 store as [128,512] column-chunks that chase the
    dep groups (first store ~3us after dep g0 lands); rows 512+ store
    as full [128,2048] rows once all of depT is resident.
  * Stores go on the sync (SP) HWDGE ring -- independent of the loads.
  * Transpose collects split DVE (dc0) / ACT (dc1); hs is one
    broadcast-AP DVE mul + 3D reduce per head group.
Matmuls run as float32r (1 cycle/row for moving dim >= 256).  FP32r
matmul inputs must be produced by a compute op, so matmul-feeding SBUF
tiles are float32r-typed and written by DVE/ACT copies, never by DMA.
"""

import numpy as np

import concourse.bass as bass
import concourse.tile as tile
from concourse import bacc, mybir
from concourse.bass_utils import run_bass_kernel_spmd

B, S, D = 8, 2048, 256
P = 128          # partitions
OC = 512         # matmul output free-dim chunk (one PSUM bank of fp32)
GB = 4           # row-blocks per input load group
NG = S // (P * GB)   # 4 load groups per input
NI = S // P      # 16 row blocks
NO = S // OC     # 4 output column chunks
ND = D // P      # 2 contraction chunks
F32 = mybir.dt.float32
F32R = mybir.dt.float32r

# packed const layout: eye | U0 | U1 | wh | wdT | b
C_EYE, C_U0, C_U1, C_WH, C_WDT, C_B = 0, P, P + D, P + 2 * D, P + 3 * D, P + 3 * D + ND
C_TOT = C_B + 1


def build_nc(reps=1):
    nc = bacc.Bacc("TRN2", target_bir_lowering=False, debug=False, num_devices=B)

    head_d = nc.dram_tensor("head", [S, D], F32, kind="ExternalInput")
    dep_d = nc.dram_tensor("dep", [S, D], F32, kind="ExternalInput")
    eye_d = nc.dram_tensor("eyep", [P, P], F32, kind="ExternalInput")
    cst_d = nc.dram_tensor("cpack", [P, C_TOT], F32, kind="ExternalInput")
    out_d = nc.dram_tensor("out", [S, S], F32, kind="ExternalOutput")

    Ident = mybir.ActivationFunctionType.Identity

    with tile.TileContext(nc) as tc:
        with (
            tc.tile_pool(name="const", bufs=1) as cpool,
            tc.tile_pool(name="persist", bufs=1) as ppool,
            tc.tile_pool(name="ttrp", bufs=2) as ttrp,
            tc.tile_pool(name="ocbuf", bufs=6) as ocbuf,
            tc.tile_pool(name="outbuf", bufs=4) as outbuf,
            tc.tile_pool(name="ps_t", bufs=2, space=bass.MemorySpace.PSUM) as ps_t,
            tc.tile_pool(name="ps_mm", bufs=6, space=bass.MemorySpace.PSUM) as ps_mm,
        ):
            # ---- all loads issued upfront on the sync HWDGE ring, in
            # critical-chain order: eye, head g0, dep g0, consts, dep g1..g3,
            # head g1..g3 ----
            eye_t = cpool.tile([P, P], F32, name="eye", tag="eye")
            nc.sync.dma_start(eye_t[:], eye_d[:])

            def load_group(src_dram, name):
                t = ppool.tile([P, GB * D], F32, name=name, tag=name)
                nc.sync.dma_start(
                    t[:].rearrange("p (j d) -> p j d", d=D),
                    src_dram.rearrange("(j p) d -> p j d", p=P),
                )
                return t

            nat_h = [None] * NG
            nat_d = [None] * NG
            nat_h[0] = load_group(head_d[0:GB * P, :], "nath0")
            nat_d[0] = load_group(dep_d[0:GB * P, :], "natd0")
            cst = cpool.tile([P, C_TOT], F32, name="cst", tag="cst")
            nc.sync.dma_start(cst[:], cst_d[:])
            for g in range(1, NG):
                nat_d[g] = load_group(dep_d[g * GB * P:(g + 1) * GB * P, :],
                                      f"natd{g}")
            for g in range(1, NG):
                nat_h[g] = load_group(head_d[g * GB * P:(g + 1) * GB * P, :],
                                      f"nath{g}")

            # ---- f32r copies of U (DVE, right after cst lands) ----
            u_sb = []
            for dc in range(ND):
                u_t = cpool.tile([P, D], F32R, name=f"u{dc}", tag=f"u{dc}")
                nc.vector.tensor_copy(u_t[:], cst[:, C_U0 + dc * D:C_U0 + (dc + 1) * D])
                u_sb.append(u_t)
            eye = eye_t[:]

            # ---- persistent SBUF tensors ----
            headT = [ppool.tile([P, S], F32R, name=f"headT{dc}", tag=f"headT{dc}")
                     for dc in range(ND)]
            depT = [ppool.tile([P, S], F32R, name=f"depT{dc}", tag=f"depT{dc}")
                    for dc in range(ND)]
            atf = [ppool.tile([P, S], F32R, name=f"atf{eb}", tag=f"atf{eb}")
                   for eb in range(ND)]
            hs_colb = ppool.tile([P, NI], F32, name="hs_colb", tag="hs_colb")

            def transpose_group(nat, dstT, g):
                # 8 PE transposes -> two [128,512] collects: dc0 DVE, dc1 ACT
                for dc in range(ND):
                    pst = ps_t.tile([P, GB * P], F32, name="pst", tag="pst")
                    for j in range(GB):
                        nc.tensor.transpose(
                            pst[:, j * P:(j + 1) * P],
                            nat[:, j * D + dc * P: j * D + dc * P + P],
                            eye,
                        )
                    dst = dstT[dc][:, g * GB * P:(g + 1) * GB * P]
                    if dc == 0:
                        nc.vector.tensor_copy(dst, pst[:])
                    else:
                        nc.scalar.copy(dst, pst[:])

            def hs_group(g):
                # hs_colb[p, g*4+j] = b + sum_d nat_h[p, j*D+d] * wh[d]
                ttr = ttrp.tile([P, GB * D], F32, name="ttr", tag="ttr")
                wh = cst[:, C_WH:C_WH + D]
                whb = bass.AP(wh.tensor, wh.offset, [wh.ap[0], [0, GB], wh.ap[1]])
                nc.vector.tensor_mul(
                    ttr[:].rearrange("p (j d) -> p j d", d=D),
                    nat_h[g][:].rearrange("p (j d) -> p j d", d=D),
                    whb,
                )
                hsr = ttrp.tile([P, GB], F32, name="hsr", tag="hsr")
                nc.vector.reduce_sum(
                    hsr[:], ttr[:].rearrange("p (j d) -> p j d", d=D),
                    axis=mybir.AxisListType.X,
                )
                nc.vector.tensor_scalar_add(
                    hs_colb[:, g * GB:(g + 1) * GB], hsr[:], cst[:, C_B:C_B + 1],
                )

            def at_chunk(g):
                # ATf for this group's 512 i-columns; eb0 copy ACT, eb1 DVE
                for eb in range(ND):
                    pa = ps_mm.tile([P, OC], F32, name="psmm", tag="psmm")
                    for dc in range(ND):
                        nc.tensor.matmul(
                            pa[:],
                            u_sb[dc][:, eb * P:(eb + 1) * P],
                            headT[dc][:, g * OC:(g + 1) * OC],
                            start=(dc == 0),
                            stop=(dc == ND - 1),
                        )
                    wdb = cst[:, C_WDT + eb:C_WDT + eb + 1]
                    if eb == 0:
                        nc.scalar.activation(
                            atf[eb][:, g * OC:(g + 1) * OC], pa[:], Ident, bias=wdb)
                    else:
                        nc.vector.tensor_scalar_add(
                            atf[eb][:, g * OC:(g + 1) * OC], pa[:], wdb)

            def mm_epi(ib, oc, dst):
                po = ps_mm.tile([P, OC], F32, name="psmm", tag="psmm")
                for eb in range(ND):
                    nc.tensor.matmul(
                        po[:],
                        atf[eb][:, ib * P:(ib + 1) * P],
                        depT[eb][:, oc * OC:(oc + 1) * OC],
                        start=(eb == 0),
                        stop=(eb == ND - 1),
                    )
                if oc == 3 or (oc == 2 and ib % 2 == 1):
                    nc.vector.tensor_scalar_add(dst, po[:], hs_colb[:, ib:ib + 1])
                else:
                    nc.scalar.activation(dst, po[:], Ident, bias=hs_colb[:, ib:ib + 1])

            def body():
                # head g0 chain first: ATf/hs for rows 0..511 ready before
                # dep g0 finishes loading
                transpose_group(nat_h[0], headT, 0)
                at_chunk(0)
                hs_group(0)
                # rows 0..511 chase the dep groups as column-chunk stores
                for g in range(NG):
                    transpose_group(nat_d[g], depT, g)
                    for ib in range(GB):
                        oct_ = ocbuf.tile([P, OC], F32, name="oct", tag="oct")
                        mm_epi(ib, g, oct_[:])
                        nc.sync.dma_start(
                            out_d[ib * P:(ib + 1) * P, g * OC:(g + 1) * OC],
                            oct_[:])
                # remaining head groups: full-row tiles (sim-time floors
                # keep their prep out of the early critical window)
                for g in range(1, NG):
                    with tc.tile_wait_until(0.006 + 0.005 * g):
                        transpose_group(nat_h[g], headT, g)
                        at_chunk(g)
                        hs_group(g)
                    for ib in range(g * GB, (g + 1) * GB):
                        ot = outbuf.tile([P, S], F32, name="ot", tag="ot")
                        for oc in range(NO):
                            mm_epi(ib, oc, ot[:, oc * OC:(oc + 1) * OC])
                        nc.sync.dma_start(out_d[ib * P:(ib + 1) * P, :], ot[:])

            if reps > 1:
                with tc.For_i(0, reps, 1):
                    body()
            else:
                body()

    nc.finalize()
    return nc


_NC_CACHE = {}


def _get_nc(reps=1):
    if reps not in _NC_CACHE:
        _NC_CACHE[reps] = build_nc(reps)
    return _NC_CACHE[reps]


def make_in_maps(head, dep, edge_U, edge_W, edge_b):
    head = np.ascontiguousarray(np.asarray(head, dtype=np.float32))
    dep = np.ascontiguousarray(np.asarray(dep, dtype=np.float32))
    u = np.asarray(edge_U, dtype=np.float32)
    w = np.asarray(edge_W, dtype=np.float32).reshape(-1)
    wh, wd = w[:D], w[D:]
    bval = float(np.asarray(edge_b).reshape(-1)[0])

    eyep = np.eye(P, dtype=np.float32)
    cpack = np.zeros((P, C_TOT), dtype=np.float32)
    cpack[:, C_U0:C_U0 + D] = u[0:P, :]
    cpack[:, C_U1:C_U1 + D] = u[P:2 * P, :]
    cpack[:, C_WH:C_WH + D] = np.tile(wh[None, :], (P, 1))
    cpack[:, C_WDT:C_WDT + ND] = wd.reshape(ND, P).T
    cpack[:, C_B] = bval
    cpack = np.ascontiguousarray(cpack)

    return [
        {"head": head[b], "dep": dep[b], "eyep": eyep, "cpack": cpack}
        for b in range(B)
    ]


def kernel(head, dep, edge_U, edge_W, edge_b):
    nc = _get_nc()
    in_maps = make_in_maps(head, dep, edge_U, edge_W, edge_b)
    res = run_bass_kernel_spmd(nc, in_maps, core_ids=list(range(B)))
    return np.stack([res.results[b]["out"] for b in range(B)], axis=0)
